# revision 33
# baseline (speedup 1.0000x reference)
"""Trainium2 Bass kernel for nn_Attention_10711648436709.

Math (faithful to reference):
    h = einsum('bhik,bhjk->bhij', Q, K) / sqrt(H)     # scale = sqrt(16) = 4
    w = softmax(h, axis=0)                            # over the BATCH axis (B=4)
    out = einsum('bhij,bhjv->bhiv', w, V)
    (mask is a no-op in the reference)

Sharding: head-parallel across 8 cores (16 heads -> 2 heads/core).
Softmax over batch stays core-local => communication-free.

Per-core layout trick: compute transposed scores S^T[j, i] so that
 - QK:  lhsT = K^T[d, j-block]  rhs = Q^T[d, i-chunk]   (host pre-transposes Q,K)
 - PV:  lhsT = V[j-block, v]    rhs = W[j, i-chunk]     (V in natural layout)
 - output accumulates as out^T[v, i] in PSUM; host transposes back.

Batch-0-pivot softmax: g_b = h_b - h_0 (b=1..3) computed by ONE full-K=128
matmul each (lhsT = [K_b^T ; K_0^T], rhs = [Q_b^T ; -Q_0^T], host packs).
Then w_b = E_b * r with E_b = e^{g_b/4}, r = 1/(1 + E_1 + E_2 + E_3), and
w_0 = r.  The whole denominator+reciprocal runs as ONE custom-DVE op
(r = 1/(1 + in0 + in1), BITWISE_NOT exponent-flip seed + 1 Newton pass,
max rel err ~2e-3) instead of the previous 5-op Newton / ACT Ln-Exp chain.
"""

import sys
import os

for p in ("/opt/trn_rl_repo",):
    if p not in sys.path:
        sys.path.insert(0, p)

import numpy as np
import ml_dtypes

B, H, S, D = 4, 16, 2048, 64
NCORES = 8
HL = H // NCORES          # 2 heads per core
NB = S // 128             # 16 j-blocks
NI = S // 512             # 4 i-chunks

TRACE = False
LAST_EXEC_NS = None
LAST_RESULTS = None

_NC = None
_RECIP_OP = None

# Chebyshev-minimax seed constants from RECIP_APPROX_FAST_CONSTS
_RC0 = -0.23549792
_RC1 = 2.0017324


def _register_recip1p():
    """Register the fused custom-DVE op  out = 1/(1 + in0 + in1).

    d = in0 + in1 + 1; seed = bitcast(~bits(d)) * c0  (exponent-flip trick,
    d > 1 always so the seed interval [-4.5,-4] for d*~d holds); one inline
    Newton pass y0*(c1 - d*y0).  7 ALU stages (<= 8 budget)."""
    global _RECIP_OP
    if _RECIP_OP is not None:
        return _RECIP_OP
    import concourse.dve_ops as dvo
    from concourse.dve_spec import (
        Spec,
        Src0,
        Src1,
        C0,
        C1,
        One,
        Bin,
        AluOp,
        lower,
        _has_src1 as has_src1,
    )
    from concourse.dve_uop import DveOpSpec

    NAME = "RECIP1P_ANT"
    if NAME in dvo._SUB_OPCODE_FOR_NAME:
        _RECIP_OP = next(o for o in dvo.OPS if o.name == NAME)
        return _RECIP_OP

    d = (Src0 + Src1) + One
    nx = Bin(AluOp.BITWISE_NOT, d, d)
    y0 = nx * C0
    y1 = y0 * (C1 - d * y0)

    def _ref(in0, in1, s0, s1, imm2):
        dd = in0.astype(np.float32) + in1.astype(np.float32) + np.float32(1.0)
        nxx = (~dd.view(np.int32)).view(np.float32)
        yy0 = nxx * np.float32(s0)
        return yy0 * (np.float32(s1) - dd * yy0)

    spec = Spec(body=y1, reference=_ref)
    row = max(dvo._SUB_OPCODE_FOR_NAME.values()) + 1
    assert row < 0x20, "custom-DVE opcode rows exhausted"
    shas = {}
    for ver in ("v3", "v4"):
        try:
            uops = lower(spec, ver=ver)
            shas[ver] = DveOpSpec(
                name=NAME, opcode=row, uops=uops, rd1_en=has_src1(spec)
            ).sha(ver)
        except Exception:
            pass
    op = dvo.DveOp(NAME, spec, subdim=False, uops_sha=shas)
    dvo.OPS.append(op)
    dvo.CUSTOM_DVE_SPECS[NAME] = spec
    dvo._SUB_OPCODE_FOR_NAME[NAME] = row
    _RECIP_OP = op
    return op


def _build_nc():
    import concourse.bass as bass
    import concourse.mybir as mybir
    import concourse.tile as tile

    DT = mybir.dt
    AF = mybir.ActivationFunctionType

    recip_op = _register_recip1p()

    nc = bass.Bass()
    ALU = mybir.AluOpType
    qt = nc.declare_dram_parameter("qt", [3, HL, 128, S], DT.bfloat16, isOutput=False)
    kt = nc.declare_dram_parameter("kt", [3, HL, 128, S], DT.bfloat16, isOutput=False)
    # host pre-swizzles V to [128, NB*D] per (b,hl) so the load is contiguous
    vv = nc.declare_dram_parameter(
        "v", [B, HL, 128, NB * D], DT.bfloat16, isOutput=False
    )
    out = nc.declare_dram_parameter("out", [B, HL, D, S], DT.float32, isOutput=True)

    with tile.TileContext(nc) as tc:
        with (
            tc.tile_pool(name="inputs", bufs=1) as ipool,
            tc.tile_pool(name="work", bufs=4) as wpool,
            tc.tile_pool(name="outsb", bufs=4) as opool,
            tc.tile_pool(name="qkps", bufs=2, space="PSUM") as qkpool,
            tc.tile_pool(name="ops", bufs=1, space="PSUM") as opsum,
        ):
            QT = ipool.tile([128, 3 * HL * S], DT.bfloat16, tag="qt")
            KT = ipool.tile([128, 3 * HL * S], DT.bfloat16, tag="kt")
            VA = ipool.tile([128, B * HL * NB * D], DT.bfloat16, tag="va")
            # Load plan: ONE ring (sync), strict need-order, so bulk
            # transfers never steal HBM bandwidth from the critical path.
            # First QK iteration only reads kt[:, :128] / qt[:, :512] (hl=0),
            # so thin fast-path slices go first; hl=1 inputs (needed ~120us
            # in) go last.
            def off_(bb, hl):
                return (bb * HL + hl) * S

            # fast-path smalls split across two DGE rings so their six
            # configs serialize ~1.8us instead of ~3.6us; everything bulky
            # stays need-ordered on sync behind the kt smalls
            for bb in range(3):
                nc.sync.dma_start(
                    out=KT[:, off_(bb, 0) : off_(bb, 0) + 256],
                    in_=kt[bb, 0, :, 0:256],
                )
            for bb in range(3):
                nc.scalar.dma_start(
                    out=QT[:, off_(bb, 0) : off_(bb, 0) + 512],
                    in_=qt[bb, 0, :, 0:512],
                )
            for bb in range(3):
                nc.sync.dma_start(
                    out=KT[:, off_(bb, 0) + 256 : off_(bb, 0) + S],
                    in_=kt[bb, 0, :, 256:S],
                )
            for b in range(B):
                voff = (b * HL + 0) * NB * D
                nc.sync.dma_start(out=VA[:, voff : voff + NB * D], in_=vv[b, 0])
            for bb in range(3):
                nc.sync.dma_start(
                    out=QT[:, off_(bb, 0) + 512 : off_(bb, 0) + S],
                    in_=qt[bb, 0, :, 512:S],
                )
            for bb in range(3):
                nc.sync.dma_start(
                    out=KT[:, off_(bb, 1) : off_(bb, 1) + S], in_=kt[bb, 1]
                )
                nc.sync.dma_start(
                    out=QT[:, off_(bb, 1) : off_(bb, 1) + S], in_=qt[bb, 1]
                )
            for b in range(B):
                voff = (b * HL + 1) * NB * D
                nc.sync.dma_start(out=VA[:, voff : voff + NB * D], in_=vv[b, 1])

            QUAD = 4  # iterations per DVE op-group (amortizes op overhead)
            NQ = NB // QUAD

            def emit_pv(item):
                # PV matmuls are emitted one quad LATE so the PE never
                # head-of-line blocks on the quad's W-mul: by the time the
                # PE reaches these, W has long completed.
                hl, ic, jq, po, rb, W = item
                Q5 = QUAD * 512
                for k in range(QUAD):
                    jb = jq * QUAD + k
                    rhss = [rb[:, k * 512 : (k + 1) * 512]] + [
                        W[:, b * Q5 + k * 512 : b * Q5 + (k + 1) * 512]
                        for b in range(3)
                    ]
                    for p in range(2):
                        for half in range(2):
                            b = 2 * p + half
                            voff = (b * HL + hl) * NB * D + jb * D
                            nc.tensor.matmul(
                                po[p][64 * half : 64 * (half + 1), :],
                                lhsT=VA[:, voff : voff + D],
                                rhs=rhss[b],
                                start=(jb == 0),
                                stop=(jb == NB - 1),
                                tile_position=(0, 64 * half),
                            )
                if jq == NQ - 1:
                    last_chunk = hl == HL - 1 and ic == NI - 1
                    for p in range(2):
                        if not last_chunk and p == 1:
                            evac_queue.append((hl, ic, po))
                            continue
                        # ACT copy per batch-pair, then 2 partition-sliced
                        # DMAs. For the final chunk, spread the work across
                        # ACT+DVE and two DMA rings to shorten the epilogue.
                        osb = opool.tile([128, 512], DT.float32, tag="osb")
                        if last_chunk and p == 1:
                            nc.vector.tensor_copy(osb, po[p])
                        else:
                            nc.scalar.copy(osb, po[p])
                        for half in range(2):
                            b = 2 * p + half
                            ring = (
                                nc.scalar
                                if (last_chunk and half == 1)
                                else nc.sync
                            )
                            ring.dma_start(
                                out=out[b, hl, :, ic * 512 : (ic + 1) * 512],
                                in_=osb[64 * half : 64 * (half + 1), :],
                            )

            pending = None
            evac_queue = []
            for hl in range(HL):
                for ic in range(NI):
                    po = [
                        opsum.tile(
                            [128, 512], DT.float32, tag=f"po{p}", name=f"po{p}"
                        )
                        for p in range(2)
                    ]
                    for jq in range(NQ):
                        # E4 layout is b-major [3, QUAD, 512] so T1/recip/W
                        # all read contiguous [128, QUAD*512] spans at 2x mode
                        E4 = wpool.tile(
                            [128, 3 * QUAD * 512], DT.bfloat16, tag="E4"
                        )
                        for k in range(QUAD):
                            jb = jq * QUAD + k
                            # [128, 3*512] = 3 banks holding g_1|g_2|g_3;
                            # bufs=2 double-buffers the QK->exp handoff
                            qk = qkpool.tile([128, 1536], DT.float32, tag="qk")
                            # QK+exp get an earlier apparent priority so the
                            # scheduler puts next-quad QKs AHEAD of the
                            # current quad's PV matmuls in the PE stream
                            # (otherwise the PE head-of-line blocks on rb/W)
                            with tc.high_priority(offset=80):
                                for bb in range(3):
                                    off = (bb * HL + hl) * S
                                    nc.tensor.matmul(
                                        qk[:, bb * 512 : (bb + 1) * 512],
                                        lhsT=KT[
                                            :,
                                            off + jb * 128 : off + jb * 128 + 128,
                                        ],
                                        rhs=QT[
                                            :,
                                            off + ic * 512 : off + ic * 512 + 512,
                                        ],
                                        start=True,
                                        stop=True,
                                    )
                                nc.scalar.activation(
                                    E4.rearrange(
                                        "q (b u n) -> q b u n", b=3, u=QUAD
                                    )[:, :, k],
                                    qk.rearrange("q (b n) -> q b n", b=3),
                                    AF.Exp,
                                    scale=0.25,
                                )
                        Q5 = QUAD * 512
                        T1 = wpool.tile([128, Q5], DT.bfloat16, tag="T1")
                        rb = wpool.tile([128, Q5], DT.bfloat16, tag="rb")
                        W = wpool.tile([128, 3 * Q5], DT.bfloat16, tag="W")
                        first_quad = hl == 0 and ic == 0 and jq == 0
                        kslices = (
                            [(k * 512, (k + 1) * 512) for k in range(QUAD)]
                            if first_quad
                            else [(0, Q5)]
                        )
                        # first quad runs per-iteration slices so the DVE
                        # starts after exp(0) instead of after 4 exps
                        for lo, hi in kslices:
                            nc.vector.tensor_add(
                                T1[:, lo:hi], E4[:, lo:hi], E4[:, Q5 + lo : Q5 + hi]
                            )
                            # r = 1/(1 + T1 + E3) in ONE fused DVE pass
                            nc.vector._custom_dve(
                                recip_op,
                                out=rb[:, lo:hi],
                                in0=T1[:, lo:hi],
                                in1=E4[:, 2 * Q5 + lo : 2 * Q5 + hi],
                                s0=_RC0,
                                s1=_RC1,
                            )
                            nc.vector.tensor_mul(
                                W.rearrange("q (b m) -> q b m", b=3)[
                                    :, :, lo:hi
                                ],
                                E4.rearrange("q (b m) -> q b m", b=3)[
                                    :, :, lo:hi
                                ],
                                rb[:, lo:hi]
                                .unsqueeze(1)
                                .broadcast_to([128, 3, hi - lo]),
                            )
                        # pop BEFORE emit_pv so an item appended by this
                        # body's emit_pv waits until the NEXT quad-body: the
                        # two chunk-end copies then spread across two quads
                        # of the exp stream instead of bunching
                        if evac_queue:
                            ehl, eic, epo = evac_queue.pop(0)
                            osb = opool.tile([128, 512], DT.float32, tag="osb")
                            nc.scalar.copy(osb, epo[1])
                            for half in range(2):
                                b = 2 + half
                                nc.sync.dma_start(
                                    out=out[b, ehl, :, eic * 512 : (eic + 1) * 512],
                                    in_=osb[64 * half : 64 * (half + 1), :],
                                )
                        if pending is not None:
                            emit_pv(pending)
                        pending = (hl, ic, jq, po, rb, W)
            emit_pv(pending)
            for ehl, eic, epo in evac_queue:
                osb = opool.tile([128, 512], DT.float32, tag="osb")
                nc.scalar.copy(osb, epo[1])
                for half in range(2):
                    b = 2 + half
                    nc.sync.dma_start(
                        out=out[b, ehl, :, eic * 512 : (eic + 1) * 512],
                        in_=osb[64 * half : 64 * (half + 1), :],
                    )

    # populate .instr bytes for InstISA subclasses (InstCustomDveAnt) — raw
    # Bass skips this pass and walrus then fails with "ISA wrong length"
    from concourse.library_overlay import lower_extended_insts

    lower_extended_insts(nc)
    return nc


def _patch_bir_waits(bir_json: bytes) -> bytes:
    """This walrus build only accepts 1 sync wait per instruction (2 for
    DMACopy); Tile emits more. Legalize:
      1. merge duplicate-semaphore waits (keep max threshold),
      2. drop waits that are transitively implied (vector-clock replay over
         the straight-line program: in-order completion per engine, FIFO per
         DMA queue, and the knowledge a producer had when it bumped a sem),
      3. split any residual multi-wait onto injected EventSemaphore
         instructions on the same engine right before the instruction.
    Only monotonic sem-inc/sem-ge-imm semaphores participate in (2); barrier
    sems (dec/eq) are left untouched."""
    import json
    from collections import defaultdict

    bir = json.loads(bir_json)

    for fn in bir["functions"]:
        insts = []
        for bb in fn["blocks"]:
            for inst in bb.get("instructions", []):
                insts.append(inst)

        # classify sems: monotonic = all updates are positive sem-inc and
        # all waits are sem-ge-imm
        bad_sems = set()
        for inst in insts:
            si = inst.get("sync_info") or {}
            for u in si.get("on_update") or []:
                if u.get("update_mode") != "sem-inc" or u.get("update_value", 0) <= 0:
                    bad_sems.add(u["id"])
            for w in si.get("on_wait") or []:
                if w.get("wait_mode") != "sem-ge-imm":
                    bad_sems.add(w["id"])

        # proc of an instruction: its engine stream, except DMACopy whose
        # completion (and sem update) is FIFO per DMA queue, keyed by the
        # sem it updates.
        def proc_of(inst):
            if inst.get("opcode") == "DMACopy":
                si = inst.get("sync_info") or {}
                ups = si.get("on_update") or []
                if ups:
                    return ("dma", ups[0]["id"])
            return ("eng", inst.get("engine"))

        sem_val = defaultdict(int)          # current cumulative value per sem
        producers = defaultdict(list)       # sem -> [(value_after, CK dict)]
        know = defaultdict(dict)            # proc -> {sem: guaranteed min}

        def join(dst, src):
            for s, v in src.items():
                if dst.get(s, 0) < v:
                    dst[s] = v

        out_blocks = {id(bb): [] for bb in fn["blocks"]}
        inj = 0
        for bb in fn["blocks"]:
            new_list = []
            for inst in bb.get("instructions", []):
                p = proc_of(inst)
                eng_p = ("eng", inst.get("engine"))
                # waits on a DMACopy are enforced by the DGE queue (FIFO per
                # queue), not the issuing engine — track knowledge per queue
                kp = p if p[0] == "dma" else eng_p
                si = inst.get("sync_info") or {}
                waits = si.get("on_wait") or []
                # merge duplicate sems
                merged = {}
                for w in waits:
                    k = w["id"]
                    if k not in merged or w.get("wait_value", 0) > merged[k].get(
                        "wait_value", 0
                    ):
                        merged[k] = w
                waits = list(merged.values())
                kept = []
                for w in waits:
                    s, v = w["id"], w.get("wait_value", 0)
                    if s in bad_sems:
                        kept.append(w)
                        continue
                    if know[kp].get(s, 0) >= v:
                        continue  # redundant
                    kept.append(w)
                    know[kp][s] = max(know[kp].get(s, 0), v)
                    # transitive knowledge from the producer that reached v
                    for val_after, ck in producers[s]:
                        if val_after >= v:
                            join(know[kp], ck)
                            break
                # split if too many waits remain
                budget = 1
                while len(kept) > budget:
                    w = kept.pop(0)
                    inj += 1
                    new_list.append(
                        {
                            "debug": inst.get("debug", 0),
                            "engine": inst.get("engine"),
                            "ins": [],
                            "name": f"WS-{inj}-{inst.get('name')}",
                            "opcode": "EventSemaphore",
                            "outs": [],
                            "sync_info": {"on_update": [], "on_wait": [w]},
                        }
                    )
                si["on_wait"] = kept
                inst["sync_info"] = si
                new_list.append(inst)
                # apply this instruction's updates for downstream knowledge
                ups = si.get("on_update") or []
                ck = None
                for u in ups:
                    s = u["id"]
                    if s in bad_sems:
                        continue
                    sem_val[s] += u.get("update_value", 0)
                    if ck is None:
                        # completion knowledge: what this proc knew here
                        # (for DMA: queue knowledge + engine state at issue)
                        ck = dict(know[kp])
                        if p[0] == "dma":
                            join(ck, know[eng_p])
                    ck[s] = sem_val[s]
                    producers[s].append((sem_val[s], ck))
                # a proc knows its own sems' values after completion
                if p[0] == "eng":
                    for u in ups:
                        if u["id"] not in bad_sems:
                            know[eng_p][u["id"]] = sem_val[u["id"]]
            out_blocks[id(bb)] = new_list
        for bb in fn["blocks"]:
            bb["instructions"] = out_blocks[id(bb)]
    return json.dumps(bir).encode()


_PATCHED = False


def _install_bir_patch():
    global _PATCHED
    if _PATCHED:
        return
    import concourse.bass2jax as bass2jax
    from concourse import bass_utils as _bu

    orig = _bu.compile_bir_kernel

    def patched(bir_json, tmpdir, neff_name="file.neff"):
        try:
            return orig(_patch_bir_waits(bir_json), tmpdir, neff_name)
        except BaseException:
            import traceback

            traceback.print_exc()
            raise

    bass2jax.compile_bir_kernel = patched
    # keep profile artifacts local — no bucket in this environment
    _bu.upload_artifacts = lambda tmpdir: str(tmpdir)
    _PATCHED = True


def _install_ntff_shim():
    """run_bass_kernel_spmd(trace=True) under axon needs
    antenv.axon_hooks.get_axon_ntff_profile_hook; the module isn't staged in
    this image, but libaxon_pjrt.so exposes the profile C ABI — recreate the
    shim (same recipe as trn_agent_boot)."""
    import sys as _sys

    if "antenv.axon_hooks" in _sys.modules:
        return
    import contextlib
    import ctypes
    import types

    import antenv  # noqa: F401

    so_path = "/opt/axon/libaxon_pjrt.so"
    hook = None
    try:
        lib = ctypes.CDLL(so_path)
        if hasattr(lib, "axon_start_nrt_profile"):
            lib.axon_start_nrt_profile.argtypes = [
                ctypes.POINTER(ctypes.c_int64),
                ctypes.c_size_t,
            ]
            lib.axon_start_nrt_profile.restype = ctypes.c_int64
            lib.axon_stop_nrt_profile.argtypes = [ctypes.c_char_p]
            lib.axon_stop_nrt_profile.restype = ctypes.c_int64

            @contextlib.contextmanager
            def hook(output_dir, device_ids):
                import jax

                jax.devices()
                if device_ids:
                    ids = (ctypes.c_int64 * len(device_ids))(*device_ids)
                    rc = lib.axon_start_nrt_profile(ids, len(device_ids))
                else:
                    rc = lib.axon_start_nrt_profile(None, 0)
                if rc != 0:
                    raise RuntimeError(f"axon_start_nrt_profile rc={rc}")
                try:
                    yield
                finally:
                    n = lib.axon_stop_nrt_profile(str(output_dir).encode())
                    print(
                        f"ntff profile: {n} file(s) -> {output_dir}",
                        file=_sys.stderr,
                    )
    except OSError:
        pass

    mod = types.ModuleType("antenv.axon_hooks")
    mod.get_axon_ntff_profile_hook = lambda: hook
    mod.set_axon_ntff_profile_hook = lambda h: None
    _sys.modules["antenv.axon_hooks"] = mod
    import antenv as _ae

    _ae.axon_hooks = mod


def kernel(query, key, value, mask=None):
    global _NC, LAST_EXEC_NS, LAST_RESULTS
    from concourse.bass_utils import run_bass_kernel_spmd

    _install_bir_patch()
    if TRACE:
        _install_ntff_shim()

    query = np.asarray(query, dtype=np.float32)
    key = np.asarray(key, dtype=np.float32)
    value = np.asarray(value, dtype=np.float32)

    if _NC is None:
        _NC = _build_nc()
    nc = _NC

    bf16 = ml_dtypes.bfloat16

    def pack_pivot(x, negate_base):
        # [B, HL, S, D] -> [B, HL, D, S]; stack [x_b^T ; (+-)x_0^T] on the
        # partition axis for b = 1..3 -> [3, HL, 128, S]
        xt = x.transpose(0, 1, 3, 2)  # [B, HL, D, S]
        base = -xt[0] if negate_base else xt[0]  # [HL, D, S]
        stk = np.stack(
            [np.concatenate([xt[b], base], axis=1) for b in (1, 2, 3)], axis=0
        )
        return np.ascontiguousarray(stk).astype(bf16)

    in_maps = []
    for c in range(NCORES):
        hs = slice(HL * c, HL * (c + 1))
        qt = pack_pivot(query[:, hs], negate_base=True)
        kt = pack_pivot(key[:, hs], negate_base=False)
        # V swizzle: [B,HL,S,D] -> [B,HL,128,NB*D] with S = NB blocks of 128
        # rows, so the device sees partition-major contiguous loads
        vc = (
            value[:, hs]
            .reshape(B, HL, NB, 128, D)
            .transpose(0, 1, 3, 2, 4)
            .reshape(B, HL, 128, NB * D)
        )
        vc = np.ascontiguousarray(vc).astype(bf16)
        in_maps.append({"qt": qt, "kt": kt, "v": vc})

    res = run_bass_kernel_spmd(
        nc, in_maps, core_ids=list(range(NCORES)), trace=TRACE
    )
    LAST_RESULTS = res
    LAST_EXEC_NS = getattr(res, "exec_time_ns", None)

    full = np.empty((B, H, S, D), dtype=np.float32)
    for c in range(NCORES):
        o = np.asarray(res.results[c]["out"])  # [B, HL, D, S]
        full[:, HL * c : HL * (c + 1)] = o.transpose(0, 1, 3, 2)
    return full


# revision 34
# speedup vs baseline: 1.0017x; 1.0017x over previous
"""Trainium2 Bass kernel for nn_Attention_10711648436709.

Math (faithful to reference):
    h = einsum('bhik,bhjk->bhij', Q, K) / sqrt(H)     # scale = sqrt(16) = 4
    w = softmax(h, axis=0)                            # over the BATCH axis (B=4)
    out = einsum('bhij,bhjv->bhiv', w, V)
    (mask is a no-op in the reference)

Sharding: head-parallel across 8 cores (16 heads -> 2 heads/core).
Softmax over batch stays core-local => communication-free.

Per-core layout trick: compute transposed scores S^T[j, i] so that
 - QK:  lhsT = K^T[d, j-block]  rhs = Q^T[d, i-chunk]   (host pre-transposes Q,K)
 - PV:  lhsT = V[j-block, v]    rhs = W[j, i-chunk]     (V in natural layout)
 - output accumulates as out^T[v, i] in PSUM; host transposes back.

Batch-0-pivot softmax: g_b = h_b - h_0 (b=1..3) computed by ONE full-K=128
matmul each (lhsT = [K_b^T ; K_0^T], rhs = [Q_b^T ; -Q_0^T], host packs).
Then w_b = E_b * r with E_b = e^{g_b/4}, r = 1/(1 + E_1 + E_2 + E_3), and
w_0 = r.  The whole denominator+reciprocal runs as ONE custom-DVE op
(r = 1/(1 + in0 + in1), BITWISE_NOT exponent-flip seed + 1 Newton pass,
max rel err ~2e-3) instead of the previous 5-op Newton / ACT Ln-Exp chain.
"""

import sys
import os

for p in ("/opt/trn_rl_repo",):
    if p not in sys.path:
        sys.path.insert(0, p)

import numpy as np
import ml_dtypes

B, H, S, D = 4, 16, 2048, 64
NCORES = 8
HL = H // NCORES          # 2 heads per core
NB = S // 128             # 16 j-blocks
NI = S // 512             # 4 i-chunks

TRACE = False
LAST_EXEC_NS = None
LAST_RESULTS = None

_NC = None
_RECIP_OP = None

# Chebyshev-minimax seed constants from RECIP_APPROX_FAST_CONSTS
_RC0 = -0.23549792
_RC1 = 2.0017324


def _register_recip1p():
    """Register the fused custom-DVE op  out = 1/(1 + in0 + in1).

    d = in0 + in1 + 1; seed = bitcast(~bits(d)) * c0  (exponent-flip trick,
    d > 1 always so the seed interval [-4.5,-4] for d*~d holds); one inline
    Newton pass y0*(c1 - d*y0).  7 ALU stages (<= 8 budget)."""
    global _RECIP_OP
    if _RECIP_OP is not None:
        return _RECIP_OP
    import concourse.dve_ops as dvo
    from concourse.dve_spec import (
        Spec,
        Src0,
        Src1,
        C0,
        C1,
        One,
        Bin,
        AluOp,
        lower,
        _has_src1 as has_src1,
    )
    from concourse.dve_uop import DveOpSpec

    NAME = "RECIP1P_ANT"
    if NAME in dvo._SUB_OPCODE_FOR_NAME:
        _RECIP_OP = next(o for o in dvo.OPS if o.name == NAME)
        return _RECIP_OP

    d = (Src0 + Src1) + One
    nx = Bin(AluOp.BITWISE_NOT, d, d)
    y0 = nx * C0
    y1 = y0 * (C1 - d * y0)

    def _ref(in0, in1, s0, s1, imm2):
        dd = in0.astype(np.float32) + in1.astype(np.float32) + np.float32(1.0)
        nxx = (~dd.view(np.int32)).view(np.float32)
        yy0 = nxx * np.float32(s0)
        return yy0 * (np.float32(s1) - dd * yy0)

    spec = Spec(body=y1, reference=_ref)
    row = max(dvo._SUB_OPCODE_FOR_NAME.values()) + 1
    assert row < 0x20, "custom-DVE opcode rows exhausted"
    shas = {}
    for ver in ("v3", "v4"):
        try:
            uops = lower(spec, ver=ver)
            shas[ver] = DveOpSpec(
                name=NAME, opcode=row, uops=uops, rd1_en=has_src1(spec)
            ).sha(ver)
        except Exception:
            pass
    op = dvo.DveOp(NAME, spec, subdim=False, uops_sha=shas)
    dvo.OPS.append(op)
    dvo.CUSTOM_DVE_SPECS[NAME] = spec
    dvo._SUB_OPCODE_FOR_NAME[NAME] = row
    _RECIP_OP = op
    return op


def _build_nc():
    import concourse.bass as bass
    import concourse.mybir as mybir
    import concourse.tile as tile

    DT = mybir.dt
    AF = mybir.ActivationFunctionType

    recip_op = _register_recip1p()

    nc = bass.Bass()
    ALU = mybir.AluOpType
    qt = nc.declare_dram_parameter("qt", [3, HL, 128, S], DT.bfloat16, isOutput=False)
    kt = nc.declare_dram_parameter("kt", [3, HL, 128, S], DT.bfloat16, isOutput=False)
    # host pre-swizzles V to [128, NB*D] per (b,hl) so the load is contiguous
    vv = nc.declare_dram_parameter(
        "v", [B, HL, 128, NB * D], DT.bfloat16, isOutput=False
    )
    out = nc.declare_dram_parameter("out", [B, HL, D, S], DT.float32, isOutput=True)

    with tile.TileContext(nc) as tc:
        with (
            tc.tile_pool(name="inputs", bufs=1) as ipool,
            tc.tile_pool(name="work", bufs=4) as wpool,
            tc.tile_pool(name="outsb", bufs=4) as opool,
            tc.tile_pool(name="qkps", bufs=2, space="PSUM") as qkpool,
            tc.tile_pool(name="ops", bufs=1, space="PSUM") as opsum,
        ):
            QT = ipool.tile([128, 3 * HL * S], DT.bfloat16, tag="qt")
            KT = ipool.tile([128, 3 * HL * S], DT.bfloat16, tag="kt")
            VA = ipool.tile([128, B * HL * NB * D], DT.bfloat16, tag="va")
            # Load plan: ONE ring (sync), strict need-order, so bulk
            # transfers never steal HBM bandwidth from the critical path.
            # First QK iteration only reads kt[:, :128] / qt[:, :512] (hl=0),
            # so thin fast-path slices go first; hl=1 inputs (needed ~120us
            # in) go last.
            def off_(bb, hl):
                return (bb * HL + hl) * S

            # fast-path smalls split across two DGE rings so their six
            # configs serialize ~1.8us instead of ~3.6us; everything bulky
            # stays need-ordered on sync behind the kt smalls
            for bb in range(3):
                nc.sync.dma_start(
                    out=KT[:, off_(bb, 0) : off_(bb, 0) + 256],
                    in_=kt[bb, 0, :, 0:256],
                )
            for bb in range(3):
                nc.scalar.dma_start(
                    out=QT[:, off_(bb, 0) : off_(bb, 0) + 512],
                    in_=qt[bb, 0, :, 0:512],
                )
            for bb in range(3):
                nc.sync.dma_start(
                    out=KT[:, off_(bb, 0) + 256 : off_(bb, 0) + S],
                    in_=kt[bb, 0, :, 256:S],
                )
            for b in range(B):
                voff = (b * HL + 0) * NB * D
                nc.sync.dma_start(out=VA[:, voff : voff + NB * D], in_=vv[b, 0])
            for bb in range(3):
                nc.sync.dma_start(
                    out=QT[:, off_(bb, 0) + 512 : off_(bb, 0) + S],
                    in_=qt[bb, 0, :, 512:S],
                )
            for bb in range(3):
                nc.sync.dma_start(
                    out=KT[:, off_(bb, 1) : off_(bb, 1) + S], in_=kt[bb, 1]
                )
                nc.sync.dma_start(
                    out=QT[:, off_(bb, 1) : off_(bb, 1) + S], in_=qt[bb, 1]
                )
            for b in range(B):
                voff = (b * HL + 1) * NB * D
                nc.sync.dma_start(out=VA[:, voff : voff + NB * D], in_=vv[b, 1])

            QUAD = 4  # iterations per DVE op-group (amortizes op overhead)
            NQ = NB // QUAD

            def emit_pv(item):
                # PV matmuls are emitted one quad LATE so the PE never
                # head-of-line blocks on the quad's W-mul: by the time the
                # PE reaches these, W has long completed.
                hl, ic, jq, po, rb, W = item
                Q5 = QUAD * 512
                for k in range(QUAD):
                    jb = jq * QUAD + k
                    rhss = [rb[:, k * 512 : (k + 1) * 512]] + [
                        W[:, b * Q5 + k * 512 : b * Q5 + (k + 1) * 512]
                        for b in range(3)
                    ]
                    for p in range(2):
                        for half in range(2):
                            b = 2 * p + half
                            voff = (b * HL + hl) * NB * D + jb * D
                            nc.tensor.matmul(
                                po[p][64 * half : 64 * (half + 1), :],
                                lhsT=VA[:, voff : voff + D],
                                rhs=rhss[b],
                                start=(jb == 0),
                                stop=(jb == NB - 1),
                                tile_position=(0, 64 * half),
                            )
                if jq == NQ - 1:
                    last_chunk = hl == HL - 1 and ic == NI - 1
                    for p in range(2):
                        if not last_chunk and p == 1:
                            evac_queue.append((hl, ic, po))
                            continue
                        # ACT copy per batch-pair, then 2 partition-sliced
                        # DMAs. For the final chunk, spread the work across
                        # ACT+DVE and two DMA rings to shorten the epilogue.
                        osb = opool.tile([128, 512], DT.float32, tag="osb")
                        if last_chunk and p == 1:
                            nc.vector.tensor_copy(osb, po[p])
                        else:
                            nc.scalar.copy(osb, po[p])
                        for half in range(2):
                            b = 2 * p + half
                            ring = (
                                nc.scalar
                                if (last_chunk and half == 1)
                                else nc.sync
                            )
                            ring.dma_start(
                                out=out[b, hl, :, ic * 512 : (ic + 1) * 512],
                                in_=osb[64 * half : 64 * (half + 1), :],
                            )

            pending = None
            evac_queue = []
            for hl in range(HL):
                for ic in range(NI):
                    po = [
                        opsum.tile(
                            [128, 512], DT.float32, tag=f"po{p}", name=f"po{p}"
                        )
                        for p in range(2)
                    ]
                    for jq in range(NQ):
                        # E4 layout is b-major [3, QUAD, 512] so T1/recip/W
                        # all read contiguous [128, QUAD*512] spans at 2x mode
                        E4 = wpool.tile(
                            [128, 3 * QUAD * 512], DT.bfloat16, tag="E4"
                        )
                        for k in range(QUAD):
                            jb = jq * QUAD + k
                            # [128, 3*512] = 3 banks holding g_1|g_2|g_3;
                            # bufs=2 double-buffers the QK->exp handoff
                            qk = qkpool.tile([128, 1536], DT.float32, tag="qk")
                            # QK+exp get an earlier apparent priority so the
                            # scheduler puts next-quad QKs AHEAD of the
                            # current quad's PV matmuls in the PE stream
                            # (otherwise the PE head-of-line blocks on rb/W)
                            with tc.high_priority(offset=80):
                                for bb in range(3):
                                    off = (bb * HL + hl) * S
                                    nc.tensor.matmul(
                                        qk[:, bb * 512 : (bb + 1) * 512],
                                        lhsT=KT[
                                            :,
                                            off + jb * 128 : off + jb * 128 + 128,
                                        ],
                                        rhs=QT[
                                            :,
                                            off + ic * 512 : off + ic * 512 + 512,
                                        ],
                                        start=True,
                                        stop=True,
                                    )
                                nc.scalar.activation(
                                    E4.rearrange(
                                        "q (b u n) -> q b u n", b=3, u=QUAD
                                    )[:, :, k],
                                    qk.rearrange("q (b n) -> q b n", b=3),
                                    AF.Exp,
                                    scale=0.25,
                                )
                        Q5 = QUAD * 512
                        T1 = wpool.tile([128, Q5], DT.bfloat16, tag="T1")
                        rb = wpool.tile([128, Q5], DT.bfloat16, tag="rb")
                        W = wpool.tile([128, 3 * Q5], DT.bfloat16, tag="W")
                        first_quad = hl == 0 and ic == 0 and jq == 0
                        kslices = (
                            [(k * 512, (k + 1) * 512) for k in range(QUAD)]
                            if first_quad
                            else [(0, Q5)]
                        )
                        # first quad runs per-iteration slices so the DVE
                        # starts after exp(0) instead of after 4 exps
                        for lo, hi in kslices:
                            nc.vector.tensor_add(
                                T1[:, lo:hi], E4[:, lo:hi], E4[:, Q5 + lo : Q5 + hi]
                            )
                            # r = 1/(1 + T1 + E3) in ONE fused DVE pass
                            nc.vector._custom_dve(
                                recip_op,
                                out=rb[:, lo:hi],
                                in0=T1[:, lo:hi],
                                in1=E4[:, 2 * Q5 + lo : 2 * Q5 + hi],
                                s0=_RC0,
                                s1=_RC1,
                            )
                            nc.vector.tensor_mul(
                                W.rearrange("q (b m) -> q b m", b=3)[
                                    :, :, lo:hi
                                ],
                                E4.rearrange("q (b m) -> q b m", b=3)[
                                    :, :, lo:hi
                                ],
                                rb[:, lo:hi]
                                .unsqueeze(1)
                                .broadcast_to([128, 3, hi - lo]),
                            )
                        if pending is not None:
                            emit_pv(pending)
                        if evac_queue:
                            ehl, eic, epo = evac_queue.pop(0)
                            osb = opool.tile([128, 512], DT.float32, tag="osb")
                            nc.scalar.copy(osb, epo[1])
                            for half in range(2):
                                b = 2 + half
                                nc.sync.dma_start(
                                    out=out[b, ehl, :, eic * 512 : (eic + 1) * 512],
                                    in_=osb[64 * half : 64 * (half + 1), :],
                                )
                        pending = (hl, ic, jq, po, rb, W)
            emit_pv(pending)

    # populate .instr bytes for InstISA subclasses (InstCustomDveAnt) — raw
    # Bass skips this pass and walrus then fails with "ISA wrong length"
    from concourse.library_overlay import lower_extended_insts

    lower_extended_insts(nc)
    return nc


def _patch_bir_waits(bir_json: bytes) -> bytes:
    """This walrus build only accepts 1 sync wait per instruction (2 for
    DMACopy); Tile emits more. Legalize:
      1. merge duplicate-semaphore waits (keep max threshold),
      2. drop waits that are transitively implied (vector-clock replay over
         the straight-line program: in-order completion per engine, FIFO per
         DMA queue, and the knowledge a producer had when it bumped a sem),
      3. split any residual multi-wait onto injected EventSemaphore
         instructions on the same engine right before the instruction.
    Only monotonic sem-inc/sem-ge-imm semaphores participate in (2); barrier
    sems (dec/eq) are left untouched."""
    import json
    from collections import defaultdict

    bir = json.loads(bir_json)

    for fn in bir["functions"]:
        insts = []
        for bb in fn["blocks"]:
            for inst in bb.get("instructions", []):
                insts.append(inst)

        # classify sems: monotonic = all updates are positive sem-inc and
        # all waits are sem-ge-imm
        bad_sems = set()
        for inst in insts:
            si = inst.get("sync_info") or {}
            for u in si.get("on_update") or []:
                if u.get("update_mode") != "sem-inc" or u.get("update_value", 0) <= 0:
                    bad_sems.add(u["id"])
            for w in si.get("on_wait") or []:
                if w.get("wait_mode") != "sem-ge-imm":
                    bad_sems.add(w["id"])

        # proc of an instruction: its engine stream, except DMACopy whose
        # completion (and sem update) is FIFO per DMA queue, keyed by the
        # sem it updates.
        def proc_of(inst):
            if inst.get("opcode") == "DMACopy":
                si = inst.get("sync_info") or {}
                ups = si.get("on_update") or []
                if ups:
                    return ("dma", ups[0]["id"])
            return ("eng", inst.get("engine"))

        sem_val = defaultdict(int)          # current cumulative value per sem
        producers = defaultdict(list)       # sem -> [(value_after, CK dict)]
        know = defaultdict(dict)            # proc -> {sem: guaranteed min}

        def join(dst, src):
            for s, v in src.items():
                if dst.get(s, 0) < v:
                    dst[s] = v

        out_blocks = {id(bb): [] for bb in fn["blocks"]}
        inj = 0
        for bb in fn["blocks"]:
            new_list = []
            for inst in bb.get("instructions", []):
                p = proc_of(inst)
                eng_p = ("eng", inst.get("engine"))
                # waits on a DMACopy are enforced by the DGE queue (FIFO per
                # queue), not the issuing engine — track knowledge per queue
                kp = p if p[0] == "dma" else eng_p
                si = inst.get("sync_info") or {}
                waits = si.get("on_wait") or []
                # merge duplicate sems
                merged = {}
                for w in waits:
                    k = w["id"]
                    if k not in merged or w.get("wait_value", 0) > merged[k].get(
                        "wait_value", 0
                    ):
                        merged[k] = w
                waits = list(merged.values())
                kept = []
                for w in waits:
                    s, v = w["id"], w.get("wait_value", 0)
                    if s in bad_sems:
                        kept.append(w)
                        continue
                    if know[kp].get(s, 0) >= v:
                        continue  # redundant
                    kept.append(w)
                    know[kp][s] = max(know[kp].get(s, 0), v)
                    # transitive knowledge from the producer that reached v
                    for val_after, ck in producers[s]:
                        if val_after >= v:
                            join(know[kp], ck)
                            break
                # split if too many waits remain
                budget = 1
                while len(kept) > budget:
                    w = kept.pop(0)
                    inj += 1
                    new_list.append(
                        {
                            "debug": inst.get("debug", 0),
                            "engine": inst.get("engine"),
                            "ins": [],
                            "name": f"WS-{inj}-{inst.get('name')}",
                            "opcode": "EventSemaphore",
                            "outs": [],
                            "sync_info": {"on_update": [], "on_wait": [w]},
                        }
                    )
                si["on_wait"] = kept
                inst["sync_info"] = si
                new_list.append(inst)
                # apply this instruction's updates for downstream knowledge
                ups = si.get("on_update") or []
                ck = None
                for u in ups:
                    s = u["id"]
                    if s in bad_sems:
                        continue
                    sem_val[s] += u.get("update_value", 0)
                    if ck is None:
                        # completion knowledge: what this proc knew here
                        # (for DMA: queue knowledge + engine state at issue)
                        ck = dict(know[kp])
                        if p[0] == "dma":
                            join(ck, know[eng_p])
                    ck[s] = sem_val[s]
                    producers[s].append((sem_val[s], ck))
                # a proc knows its own sems' values after completion
                if p[0] == "eng":
                    for u in ups:
                        if u["id"] not in bad_sems:
                            know[eng_p][u["id"]] = sem_val[u["id"]]
            out_blocks[id(bb)] = new_list
        for bb in fn["blocks"]:
            bb["instructions"] = out_blocks[id(bb)]
    return json.dumps(bir).encode()


_PATCHED = False


def _install_bir_patch():
    global _PATCHED
    if _PATCHED:
        return
    import concourse.bass2jax as bass2jax
    from concourse import bass_utils as _bu

    orig = _bu.compile_bir_kernel

    def patched(bir_json, tmpdir, neff_name="file.neff"):
        try:
            return orig(_patch_bir_waits(bir_json), tmpdir, neff_name)
        except BaseException:
            import traceback

            traceback.print_exc()
            raise

    bass2jax.compile_bir_kernel = patched
    # keep profile artifacts local — no bucket in this environment
    _bu.upload_artifacts = lambda tmpdir: str(tmpdir)
    _PATCHED = True


def _install_ntff_shim():
    """run_bass_kernel_spmd(trace=True) under axon needs
    antenv.axon_hooks.get_axon_ntff_profile_hook; the module isn't staged in
    this image, but libaxon_pjrt.so exposes the profile C ABI — recreate the
    shim (same recipe as trn_agent_boot)."""
    import sys as _sys

    if "antenv.axon_hooks" in _sys.modules:
        return
    import contextlib
    import ctypes
    import types

    import antenv  # noqa: F401

    so_path = "/opt/axon/libaxon_pjrt.so"
    hook = None
    try:
        lib = ctypes.CDLL(so_path)
        if hasattr(lib, "axon_start_nrt_profile"):
            lib.axon_start_nrt_profile.argtypes = [
                ctypes.POINTER(ctypes.c_int64),
                ctypes.c_size_t,
            ]
            lib.axon_start_nrt_profile.restype = ctypes.c_int64
            lib.axon_stop_nrt_profile.argtypes = [ctypes.c_char_p]
            lib.axon_stop_nrt_profile.restype = ctypes.c_int64

            @contextlib.contextmanager
            def hook(output_dir, device_ids):
                import jax

                jax.devices()
                if device_ids:
                    ids = (ctypes.c_int64 * len(device_ids))(*device_ids)
                    rc = lib.axon_start_nrt_profile(ids, len(device_ids))
                else:
                    rc = lib.axon_start_nrt_profile(None, 0)
                if rc != 0:
                    raise RuntimeError(f"axon_start_nrt_profile rc={rc}")
                try:
                    yield
                finally:
                    n = lib.axon_stop_nrt_profile(str(output_dir).encode())
                    print(
                        f"ntff profile: {n} file(s) -> {output_dir}",
                        file=_sys.stderr,
                    )
    except OSError:
        pass

    mod = types.ModuleType("antenv.axon_hooks")
    mod.get_axon_ntff_profile_hook = lambda: hook
    mod.set_axon_ntff_profile_hook = lambda h: None
    _sys.modules["antenv.axon_hooks"] = mod
    import antenv as _ae

    _ae.axon_hooks = mod


def kernel(query, key, value, mask=None):
    global _NC, LAST_EXEC_NS, LAST_RESULTS
    from concourse.bass_utils import run_bass_kernel_spmd

    _install_bir_patch()
    if TRACE:
        _install_ntff_shim()

    query = np.asarray(query, dtype=np.float32)
    key = np.asarray(key, dtype=np.float32)
    value = np.asarray(value, dtype=np.float32)

    if _NC is None:
        _NC = _build_nc()
    nc = _NC

    bf16 = ml_dtypes.bfloat16

    def pack_pivot(x, negate_base):
        # [B, HL, S, D] -> [B, HL, D, S]; stack [x_b^T ; (+-)x_0^T] on the
        # partition axis for b = 1..3 -> [3, HL, 128, S]
        xt = x.transpose(0, 1, 3, 2)  # [B, HL, D, S]
        base = -xt[0] if negate_base else xt[0]  # [HL, D, S]
        stk = np.stack(
            [np.concatenate([xt[b], base], axis=1) for b in (1, 2, 3)], axis=0
        )
        return np.ascontiguousarray(stk).astype(bf16)

    in_maps = []
    for c in range(NCORES):
        hs = slice(HL * c, HL * (c + 1))
        qt = pack_pivot(query[:, hs], negate_base=True)
        kt = pack_pivot(key[:, hs], negate_base=False)
        # V swizzle: [B,HL,S,D] -> [B,HL,128,NB*D] with S = NB blocks of 128
        # rows, so the device sees partition-major contiguous loads
        vc = (
            value[:, hs]
            .reshape(B, HL, NB, 128, D)
            .transpose(0, 1, 3, 2, 4)
            .reshape(B, HL, 128, NB * D)
        )
        vc = np.ascontiguousarray(vc).astype(bf16)
        in_maps.append({"qt": qt, "kt": kt, "v": vc})

    res = run_bass_kernel_spmd(
        nc, in_maps, core_ids=list(range(NCORES)), trace=TRACE
    )
    LAST_RESULTS = res
    LAST_EXEC_NS = getattr(res, "exec_time_ns", None)

    full = np.empty((B, H, S, D), dtype=np.float32)
    for c in range(NCORES):
        o = np.asarray(res.results[c]["out"])  # [B, HL, D, S]
        full[:, HL * c : HL * (c + 1)] = o.transpose(0, 1, 3, 2)
    return full
